# revision 22
# baseline (speedup 1.0000x reference)
# CVQVAE kernel for 8 trn2 NeuronCores.
#
# Sharding: 8 cores = 4 batch images x 2 vertical halves. Every core runs the
# SAME program computing the "top half" of an image; bottom-half cores receive
# vertically flipped inputs and row-flipped conv kernels (convolution commutes
# with vertical flip when the kernel rows are flipped too), and the host
# un-flips their outputs. Each core:
#   encoder convs (3->4->8->16, VALID)  -> z slice [16, 64, 122]
#   VQ: scores = 2 z.e - |e|^2 via PE matmul, exact fp32 argmax via
#       DVE max/max_index  -> idx [64*122]
#   zq gather from codebook via gpsimd ap_gather
#   decoder transpose convs -> y slice [3, 64, 128]
# No collectives; host concatenates the 8 slices.

import numpy as np

ALPHA = 1.6732632423543772
LAMBDA = 1.0507009873554805

B, C_IN, H, W = 4, 3, 128, 128
K_CODES, D = 8192, 16

ZROWS = 64          # z rows computed per core (61 owned + 3 halo)
ZW = 122
NLAT = ZROWS * 128  # z padded to width 128: block cc = padded row cc
NBLK = 64
NGRP = 8

_CACHE = {}

LCFG = [
    dict(name="l1", Cin=3, Cout=4, R=4, Hout=80, selu=True),
    dict(name="l2", Cin=4, Cout=8, R=3, Hout=72, selu=True),
    dict(name="l3", Cin=8, Cout=16, R=1, Hout=64, selu=False),
    dict(name="t1", Cin=16, Cout=8, R=1, Hout=64, selu=True, csplit=[(0, 8), (8, 16)]),
    dict(name="t2", Cin=8, Cout=4, R=1, Hout=64, selu=True),
    dict(name="t3", Cin=4, Cout=3, R=3, Hout=72, selu=False),
]


def _strips(cfg):
    """List of (row_base, n_reps) strips; each rep covers 4 output rows."""
    R = cfg["R"]
    assert cfg["Hout"] % (4 * R) == 0
    return [(r, R) for r in range(0, cfg["Hout"], 4 * R)]


def _pack_partition(cfg, s_local, j, o):
    return 32 * s_local + cfg["R"] * o + j


def _blk_weights(w_eff, bias, cfg, lam_fold):
    Cout = cfg["Cout"]
    R = cfg["R"]
    if lam_fold:
        w_eff = w_eff * np.float32(LAMBDA)
    pieces = []
    for (c0, c1) in cfg.get("csplit", [(0, cfg["Cin"])]):
        nc_ = c1 - c0
        ku = nc_ * 9
        wblk = np.zeros((ku * R, Cout * R), np.float32)
        for j in range(R):
            for o in range(Cout):
                for ci in range(nc_):
                    for di in range(3):
                        for dj in range(3):
                            k = (di * 3 + dj) * nc_ + ci
                            wblk[ku * j + k, R * o + j] = w_eff[o, c0 + ci, di, dj]
        pieces.append(wblk)
    bias_packed = np.zeros((128, 1), np.float32)
    for sl in range(4):
        for j in range(R):
            for o in range(Cout):
                bias_packed[_pack_partition(cfg, sl, j, o), 0] = bias[o]
    return pieces, bias_packed


def _host_prep(inputs):
    x = np.asarray(inputs["x"], np.float32)
    cb = np.asarray(inputs["codebook"], np.float32)

    def conv_t_eff(w):
        return np.ascontiguousarray(np.flip(w, (2, 3)).transpose(1, 0, 2, 3))

    weff = {
        "l1": np.asarray(inputs["ew1"], np.float32),
        "l2": np.asarray(inputs["ew2"], np.float32),
        "l3": np.asarray(inputs["ew3"], np.float32),
        "t1": conv_t_eff(np.asarray(inputs["dw1"], np.float32)),
        "t2": conv_t_eff(np.asarray(inputs["dw2"], np.float32)),
        "t3": conv_t_eff(np.asarray(inputs["dw3"], np.float32)),
    }
    biases = {
        "l1": np.asarray(inputs["eb1"], np.float32),
        "l2": np.asarray(inputs["eb2"], np.float32),
        "l3": np.asarray(inputs["eb3"], np.float32),
        "t1": np.asarray(inputs["db1"], np.float32),
        "t2": np.asarray(inputs["db2"], np.float32),
        "t3": np.asarray(inputs["db3"], np.float32),
    }
    lam_fold = {"l1": False, "l2": True, "l3": True, "t1": False, "t2": True, "t3": True}

    rhs_ext = np.zeros((17, K_CODES), np.float32)
    rhs_ext[:16, :] = 2.0 * cb.T
    rhs_ext[16, :] = -np.sum(cb * cb, axis=1)
    cb_tiled = np.ascontiguousarray(np.tile(cb.T, (8, 1)))  # [128, 8192]

    per_core = []
    for core in range(8):
        b, half = core // 2, core % 2
        xs = np.zeros((3, 86, 128), np.float32)
        if half == 0:
            xs[:, :70, :] = x[b, :, 0:70, :]
            flip = False
        else:
            xs[:, :70, :] = x[b, :, 58:128, :][:, ::-1, :]
            flip = True
        cm = {"x_sl": np.ascontiguousarray(xs)}
        for cfg in LCFG:
            nm = cfg["name"]
            w = weff[nm]
            if flip:
                w = w[:, :, ::-1, :]
            pieces, bp = _blk_weights(np.ascontiguousarray(w), biases[nm], cfg,
                                      lam_fold[nm])
            for i, p in enumerate(pieces):
                cm[f"w_{nm}_{i}"] = np.ascontiguousarray(p)
            cm[f"bp_{nm}"] = bp
            cm[f"bn_{nm}"] = np.ascontiguousarray(-bp)
        cm["rhs_ext"] = rhs_ext
        cm["cb_tiled"] = cb_tiled
        per_core.append(cm)
    return per_core


# --------------------------------------------------------------------------


def _build_program():
    import concourse.bass as bass
    import concourse.bacc as bacc
    import concourse.mybir as mybir
    from concourse.bass_types import AP
    from concourse.tile import TileContext

    dt = mybir.dt
    f32 = dt.float32
    nc = bacc.Bacc("TRN2", target_bir_lowering=False, debug=False)

    dram = {}
    dram["x_sl"] = nc.dram_tensor("x_sl", [3, 86, 128], f32, kind="ExternalInput")
    for cfg in LCFG:
        nm = cfg["name"]
        for i, (c0, c1) in enumerate(cfg.get("csplit", [(0, cfg["Cin"])])):
            ku = (c1 - c0) * 9
            dram[f"w_{nm}_{i}"] = nc.dram_tensor(
                f"w_{nm}_{i}", [ku * cfg["R"], cfg["Cout"] * cfg["R"]], f32,
                kind="ExternalInput")
        dram[f"bp_{nm}"] = nc.dram_tensor(f"bp_{nm}", [128, 1], f32,
                                          kind="ExternalInput")
        dram[f"bn_{nm}"] = nc.dram_tensor(f"bn_{nm}", [128, 1], f32,
                                          kind="ExternalInput")
    dram["rhs_ext"] = nc.dram_tensor("rhs_ext", [17, K_CODES], f32,
                                     kind="ExternalInput")
    dram["cb_tiled"] = nc.dram_tensor("cb_tiled", [128, K_CODES], f32,
                                      kind="ExternalInput")
    y_out = nc.dram_tensor("y_out", [3, ZROWS * 128], f32, kind="ExternalOutput")
    idx_out = nc.dram_tensor("idx_out", [128, NBLK], dt.int32, kind="ExternalOutput")

    AF = mybir.ActivationFunctionType
    ALU = mybir.AluOpType

    def _h(x):
        return x.tensor if isinstance(x, AP) else x

    with TileContext(nc, pool_alloc_mode="queue") as tc:
        p_outer = tc.tile_pool(name="outer", bufs=1).__enter__()
        p_scr = tc.tile_pool(name="scr", bufs=2).__enter__()
        p_w = tc.tile_pool(name="wpool", bufs=1).__enter__()

        J_all = p_outer.tile([128, NBLK], dt.uint32, tag="J_all")

        wsb = {}
        for cfg in LCFG:
            nm = cfg["name"]
            for i, (c0, c1) in enumerate(cfg.get("csplit", [(0, cfg["Cin"])])):
                ku = (c1 - c0) * 9
                t = p_w.tile([ku * cfg["R"], cfg["Cout"] * cfg["R"]], f32,
                             tag=f"w_{nm}_{i}", name=f"wsb_{nm}_{i}")
                nc.sync.dma_start(out=t[:], in_=dram[f"w_{nm}_{i}"].ap())
                wsb[f"w_{nm}_{i}"] = t
            for pfx in ("bp", "bn"):
                t = p_w.tile([128, 1], f32, tag=f"{pfx}_{nm}", name=f"{pfx}sb_{nm}")
                nc.sync.dma_start(out=t[:], in_=dram[f"{pfx}_{nm}"].ap())
                wsb[f"{pfx}_{nm}"] = t

        # ------------------------------------------------------------------
        def build_ib(pool, src_hdl, src_part_step, cin_range, H_im, tag,
                     src_row0=0, dram_src=False):
            """im2col tensor IB [(di,dj,c), H_im*128] from a pitch-128 source.
            Row k = (di*3+dj)*ncin + c. Reads source rows src_row0+di ..
            src_row0+H_im-1+di."""
            c0, c1 = cin_range
            ncin = c1 - c0
            ib = pool.tile([ncin * 9, H_im * 128], f32, tag=tag, name=f"ib_{tag}")
            run = H_im * 128
            tc.strict_bb_all_engine_barrier()
            if dram_src:
                for di in range(3):
                    src = AP(tensor=_h(src_hdl),
                             offset=c0 * src_part_step + (src_row0 + di) * 128,
                             ap=[[1, 3], [src_part_step, ncin], [1, run]])
                    dst = AP(tensor=_h(ib), offset=3 * di * ncin * run,
                             ap=[[run, 3 * ncin], [1, run]])
                    nc.scalar.dma_start(out=dst, in_=src)
            else:
                for di in range(3):
                    for dj in range(3):
                        src = AP(tensor=_h(src_hdl),
                                 offset=c0 * src_part_step
                                 + (src_row0 + di) * 128 + dj,
                                 ap=[[src_part_step, ncin], [1, run]])
                        dst = AP(tensor=_h(ib),
                                 offset=(3 * di + dj) * ncin * run,
                                 ap=[[run, ncin], [1, run]])
                        nc.scalar.dma_start(out=dst, in_=src)
            return ib

        def build_ibr(pool, ib, cfg, strips, tag):
            """rep-replicated im2col IBR [K_unit*R, n_strips*512]."""
            ku = ib.shape[0]
            R = cfg["R"]
            ns = len(strips)
            ibr = pool.tile([ku * R, ns * 512], f32, tag=tag, name=f"ibr_{tag}")
            ibrun = ib.shape[1]
            tc.strict_bb_all_engine_barrier()
            for j in range(R):
                src = AP(tensor=_h(ib), offset=4 * j * 128,
                         ap=[[ibrun, ku], [4 * R * 128, ns], [1, 512]])
                dst = AP(tensor=_h(ibr), offset=j * ku * ns * 512,
                         ap=[[ns * 512, ku], [512, ns], [1, 512]])
                nc.scalar.dma_start(out=dst, in_=src)
            return ibr

        def selu_emit(cfg, ps, sl, emit):
            """evacuate psum tile (strips sl) + optional selu + repack."""
            nm = cfg["name"]
            asel = p_scr.tile([128, 512], f32, tag="asel", name="asel")
            if cfg["selu"]:
                n_t = p_scr.tile([128, 512], f32, tag="selu_n", name="n_t")
                e_t = p_scr.tile([128, 512], f32, tag="selu_e", name="e_t")
                r_t = p_scr.tile([128, 512], f32, tag="selu_r", name="r_t")
                nc.scalar.activation(n_t[:], ps[:], AF.Relu, scale=-1.0,
                                     bias=wsb[f"bn_{nm}"][:, 0:1])
                nc.scalar.activation(r_t[:], ps[:], AF.Relu, scale=1.0,
                                     bias=wsb[f"bp_{nm}"][:, 0:1])
                nc.scalar.activation(e_t[:], n_t[:], AF.Exp, scale=-1.0)
                nc.vector.tensor_scalar(asel[:], e_t[:], ALPHA, -ALPHA,
                                        op0=ALU.mult, op1=ALU.add)
                nc.vector.tensor_tensor(out=asel[:], in0=asel[:], in1=r_t[:],
                                        op=ALU.add)
            else:
                nc.vector.tensor_scalar(asel[:], ps[:],
                                        wsb[f"bp_{nm}"][:, 0:1], None,
                                        op0=ALU.add)
            for si, (rowb, nreps) in enumerate(sl):
                emit(asel, si, rowb, nreps)

        def conv_layer(cfg, psum_pool, rhs_source, emit):
            nm = cfg["name"]
            strips = _strips(cfg)
            Cout = cfg["Cout"]
            pieces = cfg.get("csplit", [(0, cfg["Cin"])])
            ntiles = (len(strips) + 3) // 4
            for t in range(ntiles):
                sl = strips[t * 4:(t + 1) * 4]
                ps = psum_pool.tile([128, 512], f32, tag="psc", name="ps_c")
                nc.vector.memset(ps[:], 0.0)
                for si, (rowb, nreps) in enumerate(sl):
                    for pi in range(len(pieces)):
                        rhs = rhs_source(t * 4 + si, pi, strips)
                        w = wsb[f"w_{nm}_{pi}"]
                        kk = rhs.shape[0]
                        mm = Cout * nreps
                        nc.tensor.matmul(
                            out=ps[32 * si:32 * si + mm, :],
                            lhsT=w[:kk, :mm],
                            rhs=rhs,
                            start=(pi == 0), stop=(pi == len(pieces) - 1),
                            tile_position=(0, 32 * si))
                selu_emit(cfg, ps, sl, emit)

        # ------------------------------------------------------------------
        # encoder
        p_pse = tc.tile_pool(name="pse", bufs=4, space="PSUM").__enter__()
        p_A1 = tc.tile_pool(name="pA1", bufs=1).__enter__()
        p_ib1 = tc.tile_pool(name="pib1", bufs=1).__enter__()
        A1 = p_A1.tile([4, 80 * 128], f32, tag="A1")

        cfg1 = LCFG[0]
        strips1 = _strips(cfg1)
        ib1 = build_ib(p_ib1, dram["x_sl"], 86 * 128, (0, 3), 82, "ib1", dram_src=True)
        ibr1 = build_ibr(p_ib1, ib1, cfg1, strips1, "ibr1")

        def rhs_from_ibr(ibr, ku, strips):
            ns = len(strips)

            def f(s, pi, strips_):
                return AP(tensor=_h(ibr), offset=s * 512,
                          ap=[[ns * 512, ku * strips_[s][1]], [1, 512]])
            return f

        def emit_full(A, Cout):
            Afree = A.shape[1]

            def f(asel, si, rowb, nreps):
                src = AP(tensor=_h(asel), offset=32 * si * 512,
                         ap=[[512, Cout * nreps], [1, 512]])
                dst = AP(tensor=_h(A), offset=rowb * 128,
                         ap=[[Afree, Cout], [512, nreps], [1, 512]])
                nc.scalar.dma_start(out=dst, in_=src)
            return f

        conv_layer(cfg1, p_pse, rhs_from_ibr(ibr1, 27, strips1), emit_full(A1, 4))
        p_ib1.__exit__(None, None, None)

        p_A2 = tc.tile_pool(name="pA2", bufs=1).__enter__()
        p_ib2 = tc.tile_pool(name="pib2", bufs=1).__enter__()
        A2 = p_A2.tile([8, 72 * 128], f32, tag="A2")
        cfg2 = LCFG[1]
        strips2 = _strips(cfg2)
        ib2 = build_ib(p_ib2, A1, 80 * 128, (0, 4), 74, "ib2")
        ibr2 = build_ibr(p_ib2, ib2, cfg2, strips2, "ibr2")
        conv_layer(cfg2, p_pse, rhs_from_ibr(ibr2, 36, strips2), emit_full(A2, 8))
        p_ib2.__exit__(None, None, None)
        p_A1.__exit__(None, None, None)

        # L3 -> z_ext
        p_vq = tc.tile_pool(name="pvq", bufs=1).__enter__()
        z_ext = p_vq.tile([17, NLAT], f32, tag="z_ext")
        rhs_sb = p_vq.tile([17, K_CODES], f32, tag="rhs_sb")
        nc.sync.dma_start(out=rhs_sb[:], in_=dram["rhs_ext"].ap())
        ones_sm = p_scr.tile([1, 128], f32, tag="ones_sm", bufs=1)
        nc.vector.memset(ones_sm[:], 1.0)
        nc.sync.dma_start(
            out=AP(tensor=_h(z_ext), offset=16 * NLAT,
                   ap=[[NLAT, 1], [128, NBLK], [1, 128]]),
            in_=AP(tensor=_h(ones_sm), offset=0,
                   ap=[[128, 1], [0, NBLK], [1, 128]]))

        p_ib3 = tc.tile_pool(name="pib3", bufs=1).__enter__()
        cfg3 = LCFG[2]
        strips3 = _strips(cfg3)
        ib3 = build_ib(p_ib3, A2, 72 * 128, (0, 8), 66, "ib3")

        def rhs3(s, pi, strips):
            return AP(tensor=_h(ib3), offset=strips[s][0] * 128,
                      ap=[[66 * 128, 72], [1, 512]])

        def emit_z(asel, si, rowb, nreps):
            src = AP(tensor=_h(asel), offset=32 * si * 512,
                     ap=[[512, 16], [1, 512]])
            dst = AP(tensor=_h(z_ext), offset=rowb * 128,
                     ap=[[NLAT, 16], [1, 512]])
            nc.scalar.dma_start(out=dst, in_=src)

        conv_layer(cfg3, p_pse, rhs3, emit_z)
        p_ib3.__exit__(None, None, None)
        p_A2.__exit__(None, None, None)
        p_pse.__exit__(None, None, None)

        # ------------------------------------------------------------------
        # VQ
        p_sc = tc.tile_pool(name="psc_pool", bufs=2).__enter__()
        p_psvq = tc.tile_pool(name="psvq", bufs=2, space="PSUM").__enter__()
        p_mm = tc.tile_pool(name="pmm", bufs=2).__enter__()

        tc.strict_bb_all_engine_barrier()
        for cc in range(NBLK):
            lhsT = z_ext[:, 128 * cc:128 * (cc + 1)]
            scores = p_sc.tile([128, K_CODES], f32, tag="scores", name="scores")
            for q in range(4):
                ps = p_psvq.tile([128, 2048], f32, tag="psvq", name="ps_vq")
                for k in range(4):
                    nc.tensor.matmul(
                        out=ps[:, 512 * k:512 * (k + 1)],
                        lhsT=lhsT,
                        rhs=rhs_sb[:, (4 * q + k) * 512:(4 * q + k + 1) * 512],
                        start=True, stop=True)
                nc.scalar.activation(scores[:, 2048 * q:2048 * (q + 1)], ps[:],
                                     AF.Copy)
            mx8 = p_mm.tile([128, 8], f32, tag="mx8", name="mx8")
            j8 = p_mm.tile([128, 8], dt.uint32, tag="j8", name="j8")
            nc.vector.max(out=mx8[:], in_=scores[:])
            nc.vector.max_index(j8[:], mx8[:], scores[:])
            nc.vector.tensor_copy(out=J_all[:, cc:cc + 1], in_=j8[:, 0:1])

        p_mm.__exit__(None, None, None)
        p_psvq.__exit__(None, None, None)
        p_sc.__exit__(None, None, None)
        p_vq.__exit__(None, None, None)

        J_i32 = p_scr.tile([128, NBLK], dt.int32, tag="J_i32")
        nc.vector.tensor_copy(out=J_i32[:], in_=J_all[:])
        nc.sync.dma_start(out=idx_out.ap(), in_=J_i32[:])

        # ------------------------------------------------------------------
        # gather
        p_gat = tc.tile_pool(name="pgat", bufs=1).__enter__()
        p_zqT = tc.tile_pool(name="pzqT", bufs=1).__enter__()
        C_sb8 = p_gat.tile([128, K_CODES], f32, tag="C_sb8")
        nc.sync.dma_start(out=C_sb8[:], in_=dram["cb_tiled"].ap())
        J16 = p_gat.tile([128, NBLK], dt.int16, tag="J16")
        nc.vector.tensor_copy(out=J16[:], in_=J_all[:])
        zq8 = p_gat.tile([128, 1024], f32, tag="zq8")
        from concourse import library_config as _libcfg
        nc.gpsimd.load_library(_libcfg.ap_gather)
        nc.gpsimd.ap_gather(zq8[:], C_sb8[:], J16[:], channels=128,
                            num_elems=K_CODES, d=1, num_idxs=1024)

        zqT = p_zqT.tile([16, 68 * 128], f32, tag="zqT")
        nc.vector.memset(zqT[:], 0.0)
        for g in range(8):
            njm = 16 if g < 7 else 10
            src = AP(tensor=_h(zq8), offset=16 * g * 1024,
                     ap=[[1024, 16], [16, ZROWS], [1, njm]])
            dst = AP(tensor=_h(zqT), offset=2 * 128 + 2 + 16 * g,
                     ap=[[68 * 128, 16], [128, ZROWS], [1, njm]])
            nc.sync.dma_start(out=dst, in_=src)
        p_gat.__exit__(None, None, None)

        # ------------------------------------------------------------------
        # decoder
        p_psd = tc.tile_pool(name="psd", bufs=4, space="PSUM").__enter__()
        cfg4 = LCFG[3]
        p_At1 = tc.tile_pool(name="pAt1", bufs=1).__enter__()
        At1 = p_At1.tile([8, 70 * 128], f32, tag="At1")
        nc.vector.memset(At1[:], 0.0)

        p_ibt1 = tc.tile_pool(name="pibt1", bufs=2).__enter__()

        def emit_t1(asel, si, rowb, nreps):
            src = AP(tensor=_h(asel), offset=32 * si * 512,
                     ap=[[512, 8], [128, 4], [1, 124]])
            dst = AP(tensor=_h(At1), offset=(rowb + 2) * 128 + 2,
                     ap=[[70 * 128, 8], [128, 4], [1, 124]])
            nc.scalar.dma_start(out=dst, in_=src)

        tc.strict_bb_all_engine_barrier()
        for hh in range(2):
            row0 = hh * 32
            ibs = []
            for pi, (c0, c1) in enumerate(cfg4["csplit"]):
                ibs.append(build_ib(p_ibt1, zqT, 68 * 128, (c0, c1), 32,
                                    "ibt1", src_row0=row0))
            for t in range(2):
                ps = p_psd.tile([128, 512], f32, tag="psc", name="ps_t1")
                nc.vector.memset(ps[:], 0.0)
                for si in range(4):
                    for pi in range(2):
                        rhs = AP(tensor=_h(ibs[pi]),
                                 offset=(t * 16 + si * 4) * 128,
                                 ap=[[32 * 128, 72], [1, 512]])
                        nc.tensor.matmul(
                            out=ps[32 * si:32 * si + 8, :],
                            lhsT=wsb[f"w_t1_{pi}"][:, :8],
                            rhs=rhs,
                            start=(pi == 0), stop=(pi == 1),
                            tile_position=(0, 32 * si))
                sl = [(row0 + t * 16 + si * 4, 1) for si in range(4)]
                selu_emit(cfg4, ps, sl, emit_t1)
        p_ibt1.__exit__(None, None, None)
        p_zqT.__exit__(None, None, None)

        # T2
        cfg5 = LCFG[4]
        strips5 = _strips(cfg5)
        p_At2 = tc.tile_pool(name="pAt2", bufs=1).__enter__()
        At2 = p_At2.tile([4, 78 * 128], f32, tag="At2")
        nc.vector.memset(At2[:], 0.0)
        p_ibt2 = tc.tile_pool(name="pibt2", bufs=1).__enter__()
        ibt2 = build_ib(p_ibt2, At1, 70 * 128, (0, 8), 66, "ibt2")

        def rhs5(s, pi, strips):
            return AP(tensor=_h(ibt2), offset=strips[s][0] * 128,
                      ap=[[66 * 128, 72], [1, 512]])

        def emit_t2(asel, si, rowb, nreps):
            src = AP(tensor=_h(asel), offset=32 * si * 512,
                     ap=[[512, 4], [128, 4], [1, 126]])
            dst = AP(tensor=_h(At2), offset=(rowb + 2) * 128 + 2,
                     ap=[[78 * 128, 4], [128, 4], [1, 126]])
            nc.scalar.dma_start(out=dst, in_=src)

        conv_layer(cfg5, p_psd, rhs5, emit_t2)
        p_ibt2.__exit__(None, None, None)
        p_At1.__exit__(None, None, None)

        # T3 -> y
        cfg6 = LCFG[5]
        strips6 = _strips(cfg6)
        p_ibt3 = tc.tile_pool(name="pibt3", bufs=1).__enter__()
        ibt3 = build_ib(p_ibt3, At2, 78 * 128, (0, 4), 74, "ibt3")
        ibr6 = build_ibr(p_ibt3, ibt3, cfg6, strips6, "ibr6")

        def emit_y(asel, si, rowb, nreps):
            nj = sum(1 for j in range(nreps) if rowb + 4 * j + 3 < ZROWS)
            if nj == 0:
                return
            if nj == nreps:
                src = AP(tensor=_h(asel), offset=32 * si * 512,
                         ap=[[512, 3 * nreps], [1, 512]])
                dst = AP(tensor=_h(y_out), offset=rowb * 128,
                         ap=[[ZROWS * 128, 3], [512, nreps], [1, 512]])
                nc.scalar.dma_start(out=dst, in_=src)
            else:
                for o in range(3):
                    src = AP(tensor=_h(asel),
                             offset=(32 * si + nreps * o) * 512,
                             ap=[[512, nj], [1, 512]])
                    dst = AP(tensor=_h(y_out), offset=o * ZROWS * 128 + rowb * 128,
                             ap=[[512, nj], [1, 512]])
                    nc.scalar.dma_start(out=dst, in_=src)

        conv_layer(cfg6, p_psd, rhs_from_ibr(ibr6, 36, strips6), emit_y)
        p_ibt3.__exit__(None, None, None)
        p_At2.__exit__(None, None, None)
        p_psd.__exit__(None, None, None)

        p_w.__exit__(None, None, None)
        p_scr.__exit__(None, None, None)
        p_outer.__exit__(None, None, None)

    return nc


# --------------------------------------------------------------------------


def _unpermute_idx(J):
    """J [128, 64] (partition=col, slot=row) -> [64, 122]."""
    return np.ascontiguousarray(J.T[:, :ZW])


def kernel(**inputs):
    from concourse import bass_utils

    if "nc" not in _CACHE:
        _CACHE["nc"] = _build_program()
    nc = _CACHE["nc"]

    per_core = _host_prep(inputs)
    res = bass_utils.run_bass_kernel_spmd(nc, per_core, core_ids=list(range(8)))
    _CACHE["last_result"] = res

    y = np.zeros((B, C_IN, H, W), np.float32)
    idx = np.zeros((B, ZW, ZW), np.int32)
    for core in range(8):
        b, half = core // 2, core % 2
        r = res.results[core]
        yc = r["y_out"].reshape(3, ZROWS, 128)
        ic = _unpermute_idx(r["idx_out"])
        if half == 0:
            y[b, :, 0:64, :] = yc
            idx[b, 0:61, :] = ic[0:61]
        else:
            y[b, :, 64:128, :] = yc[:, ::-1, :]
            idx[b, 61:122, :] = ic[0:61][::-1]
    return (y, idx.reshape(-1))


def bench(inputs, iters=10):
    """Time the 8-core NEFF execution with resident device inputs.
    Returns (best_wall_s, per_iter_list)."""
    import time
    import jax
    import jax.numpy as jnp
    import numpy as np_
    from jax.sharding import Mesh, PartitionSpec
    from jax.experimental.shard_map import shard_map
    import concourse.mybir as mybir
    from concourse import bass2jax
    from concourse.bass2jax import _bass_exec_p, install_neuronx_cc_hook

    install_neuronx_cc_hook()
    if "nc" not in _CACHE:
        _CACHE["nc"] = _build_program()
    nc = _CACHE["nc"]
    in_maps = _host_prep(inputs)
    n_cores = 8

    in_names, out_names, out_avals, zero_outs = [], [], [], []
    for alloc in nc.m.functions[0].allocations:
        if not isinstance(alloc, mybir.MemoryLocationSet):
            continue
        name = alloc.memorylocations[0].name
        if alloc.kind == "ExternalInput":
            if name in in_maps[0]:
                in_names.append(name)
        elif alloc.kind == "ExternalOutput":
            out_names.append(name)
            sh = tuple(alloc.tensor_shape)
            dtp = mybir.dt.np(alloc.dtype)
            out_avals.append(jax.core.ShapedArray(sh, dtp))
            zero_outs.append(np_.zeros(sh, dtp))
    in_names = [n for n in in_names if n != "partition_id"]
    n_params = len(in_names)
    n_outs = len(out_names)
    all_in_names = list(in_names) + list(out_names)

    partition_name = (nc.partition_id_tensor.name
                      if nc.partition_id_tensor else None)
    if partition_name is not None:
        all_in_names.append(partition_name)

    def _body(*args):
        operands = list(args)
        if partition_name is not None:
            operands.append(bass2jax.partition_id_tensor())
        outs = _bass_exec_p.bind(
            *operands, out_avals=tuple(out_avals), in_names=tuple(all_in_names),
            out_names=tuple(out_names), lowering_input_output_aliases=(),
            sim_require_finite=True, sim_require_nnan=True, nc=nc)
        return tuple(outs)

    devices = jax.devices()[:n_cores]
    mesh = Mesh(np_.asarray(devices), ("core",))
    in_specs = (PartitionSpec("core"),) * (n_params + n_outs)
    out_specs = (PartitionSpec("core"),) * n_outs
    donate = tuple(range(n_params, n_params + n_outs))
    sharded = jax.jit(
        shard_map(_body, mesh=mesh, in_specs=in_specs, out_specs=out_specs,
                  check_rep=False),
        donate_argnums=donate, keep_unused=True)

    concat_in = [
        np_.concatenate([np_.asarray(in_maps[c][nm]) for c in range(n_cores)], axis=0)
        for nm in in_names]
    sh_in = jax.sharding.NamedSharding(mesh, PartitionSpec("core"))
    dev_in = [jax.device_put(a, sh_in) for a in concat_in]

    def zeros():
        return [jax.device_put(jnp.zeros((n_cores * z.shape[0],) + z.shape[1:],
                                         z.dtype), sh_in) for z in zero_outs]

    # warmup
    o = sharded(*dev_in, *zeros()); jax.block_until_ready(o)
    times = []
    for _ in range(iters):
        zs = zeros()
        jax.block_until_ready(zs)
        t0 = time.perf_counter()
        o = sharded(*dev_in, *zs)
        jax.block_until_ready(o)
        times.append(time.perf_counter() - t0)
    return min(times), times
